# revision 7
# baseline (speedup 1.0000x reference)
"""Trainium2 Bass kernel for nn_CausalAttentionLayer (sparse_attention).

Reference computes, per batch b (B=32, Nq=Nk=1024, C=128, CM=256):
    S = Q @ K^T                      # [1024, 1024], no 1/sqrt(d) scale
    P = softmax(S, axis=-1) * strict_lower_mask   # mask AFTER full-row softmax
    O = P @ V                        # [1024, 256]

Sharding: data-parallel over batch, 4 batches per core on 8 NeuronCores.

Device algorithm (per batch), in the transposed S^T = K Q^T layout (k on
partitions, q on free) so no on-device transposes are needed:
  - Q^T, K^T prepared on host as [C, Nq] bf16 hi/lo pairs;
    S^T block = Kh^T Qh + Kl^T Qh + Kh^T Ql (error ~2^-18, fp32 PSUM accum)
  - P^T = exp(S^T) on ScalarE in [128, 512] halves, written float32r to SBUF
  - l[q] = sum_k exp(S[q, k]) over ALL k via ones-matmuls into PSUM [1, 1024],
    then redistributed to r = 1/l in [128, 8] via 8 tiny transpose-matmuls
  - strict-upper mask on diagonal blocks (DVE), causal PV (36 of 64 block mms,
    f32r), O *= r (per-partition scalar), DMA out.

Scheduling: batch-level software pipeline — the PV phase of batch b is
emitted AFTER the S phase of batch b+1 so the PE never waits on the
exp -> l -> 1/l chain; l ones-matmuls run one block late (lag-1) so they
never wait on ACT.
"""

import sys
from contextlib import ExitStack

import numpy as np

sys.path.insert(0, "/opt/trn_rl_repo")

import ml_dtypes  # noqa: E402

import concourse.tile as tile  # noqa: E402
from concourse import bacc, mybir  # noqa: E402
from concourse.bass_utils import run_bass_kernel_spmd  # noqa: E402

N_CORES = 8
B_TOTAL = 32
NQ = 1024
C = 128
CM = 256
NBLK = NQ // 128  # 8

_cache = {}

f32 = mybir.dt.float32
f32r = mybir.dt.float32r
bf16 = mybir.dt.bfloat16


def emit_kernel(nc, tc, ctx, aps, b_core):
    qh, ql, kh, kl, v, mask, ones, out = aps
    const_pool = ctx.enter_context(tc.tile_pool(name="const", bufs=1))
    qk_pool = ctx.enter_context(tc.tile_pool(name="qk", bufs=8))
    v_pool = ctx.enter_context(tc.tile_pool(name="vp", bufs=16))
    pt_pool = ctx.enter_context(tc.tile_pool(name="pt", bufs=16))
    ptm_pool = ctx.enter_context(tc.tile_pool(name="ptm", bufs=2))
    l_pool = ctx.enter_context(tc.tile_pool(name="lsb", bufs=2))
    r_pool = ctx.enter_context(tc.tile_pool(name="rsb", bufs=2))
    o_pool = ctx.enter_context(tc.tile_pool(name="osb", bufs=4))
    ps_s = ctx.enter_context(tc.tile_pool(name="ps_s", bufs=3, space="PSUM"))
    ps_o = ctx.enter_context(tc.tile_pool(name="ps_o", bufs=2, space="PSUM"))
    ps_l = ctx.enter_context(tc.tile_pool(name="ps_l", bufs=1, space="PSUM"))
    ps_r = ctx.enter_context(tc.tile_pool(name="ps_r", bufs=1, space="PSUM"))

    mask_sb = const_pool.tile([128, 128], f32r)
    nc.sync.dma_start(mask_sb[:], mask)
    ones_sb = const_pool.tile([128, 1], f32r)
    nc.sync.dma_start(ones_sb[:], ones)
    onesf_sb = const_pool.tile([1, 1], f32)
    nc.vector.memset(onesf_sb[:], 1.0)

    def emit_s_phase(b):
        st = {}
        qh_sb = qk_pool.tile([C, NQ], bf16, tag="qh")
        nc.sync.dma_start(qh_sb[:], qh[b, :, :])
        ql_sb = qk_pool.tile([C, NQ], bf16, tag="ql")
        nc.sync.dma_start(ql_sb[:], ql[b, :, :])
        kh_sb = qk_pool.tile([C, NQ], bf16, tag="kh")
        nc.sync.dma_start(kh_sb[:], kh[b, :, :])
        kl_sb = qk_pool.tile([C, NQ], bf16, tag="kl")
        nc.sync.dma_start(kl_sb[:], kl[b, :, :])
        st["v"] = []
        for i in range(NBLK):
            vt = v_pool.tile([128, CM], f32r, tag="v")
            nc.sync.dma_start(vt[:], v[b, 128 * i : 128 * (i + 1), :])
            st["v"].append(vt)

        psl = ps_l.tile([1, NQ], f32)
        st["psl"] = psl
        st["pt"] = []

        def l_mms(i, start):
            for h in (0, 1):
                nc.tensor.matmul(
                    psl[:, 512 * h : 512 * (h + 1)],
                    ones_sb[:],
                    st["pt"][i][:, 512 * h : 512 * (h + 1)],
                    start=start,
                    stop=(i == NBLK - 1),
                    skip_group_check=True,
                )

        for i in range(NBLK):
            kslc = slice(128 * i, 128 * (i + 1))
            pt_i = pt_pool.tile([128, NQ], f32r, tag="pt")
            for h in (0, 1):
                qslc = slice(512 * h, 512 * (h + 1))
                s_ps = ps_s.tile([128, 512], f32, tag="s")
                terms = ((kh_sb, qh_sb), (kl_sb, qh_sb), (kh_sb, ql_sb))
                for t, (kt_, qt_) in enumerate(terms):
                    nc.tensor.matmul(
                        s_ps[:], kt_[:, kslc], qt_[:, qslc],
                        start=(t == 0), stop=(t == 2),
                    )
                nc.scalar.activation(
                    pt_i[:, qslc], s_ps[:], mybir.ActivationFunctionType.Exp
                )
            st["pt"].append(pt_i)
            # lag-1: sum the previous block's exp so PE never waits on ACT
            if i > 0:
                l_mms(i - 1, start=(i == 1))
        l_mms(NBLK - 1, start=False)
        # copy l PSUM->SBUF on ACT (it has slack after the last exp)
        l_sb = l_pool.tile([1, NQ], f32, tag="l")
        nc.scalar.copy(l_sb[:], psl[:])
        st["l_sb"] = l_sb
        return st

    def emit_pv_phase(st):
        # diagonal-block masks (DVE) — inputs were ready long ago
        ptm = ptm_pool.tile([128, NQ], f32r, tag="ptm")
        for i in range(NBLK):
            kslc = slice(128 * i, 128 * (i + 1))
            nc.vector.tensor_mul(ptm[:, kslc], st["pt"][i][:, kslc], mask_sb[:])
        # l -> r = 1/l in [128, 8] layout via 8 transpose-matmuls
        psr = ps_r.tile([128, 8], f32)
        for j in range(NBLK):
            nc.tensor.matmul(
                psr[:, j : j + 1],
                st["l_sb"][:, 128 * j : 128 * (j + 1)],
                onesf_sb[0:1, :],
                start=True, stop=True,
            )
        r_sb = r_pool.tile([128, 8], f32, tag="r")
        nc.vector.reciprocal(r_sb[:], psr[:])

        for j in range(NBLK):
            jslc = slice(128 * j, 128 * (j + 1))
            o_ps = ps_o.tile([128, CM], f32, tag="o")
            for i in range(j + 1):
                lhsT = ptm[:, jslc] if i == j else st["pt"][i][:, jslc]
                nc.tensor.matmul(
                    o_ps[:], lhsT, st["v"][i][:],
                    start=(i == 0), stop=(i == j),
                )
            o_sb = o_pool.tile([128, CM], f32, tag="o_sb")
            nc.vector.tensor_scalar_mul(o_sb[:], o_ps[:], r_sb[:, j : j + 1])
            nc.sync.dma_start(out[st["b"], jslc, :], o_sb[:])

    prev = None
    for b in range(b_core):
        st = emit_s_phase(b)
        st["b"] = b
        if prev is not None:
            emit_pv_phase(prev)
        prev = st
    emit_pv_phase(prev)


def build(b_core):
    """Build + compile the per-core Bass program processing b_core batches."""
    nc = bacc.Bacc(
        "TRN2", target_bir_lowering=False, debug=False, num_devices=N_CORES
    )
    qh = nc.dram_tensor("qh", [b_core, C, NQ], bf16, kind="ExternalInput").ap()
    ql = nc.dram_tensor("ql", [b_core, C, NQ], bf16, kind="ExternalInput").ap()
    kh = nc.dram_tensor("kh", [b_core, C, NQ], bf16, kind="ExternalInput").ap()
    kl = nc.dram_tensor("kl", [b_core, C, NQ], bf16, kind="ExternalInput").ap()
    v = nc.dram_tensor("v", [b_core, NQ, CM], f32r, kind="ExternalInput").ap()
    mask = nc.dram_tensor("mask", [128, 128], f32r, kind="ExternalInput").ap()
    ones = nc.dram_tensor("ones", [128, 1], f32r, kind="ExternalInput").ap()
    out = nc.dram_tensor("out", [b_core, NQ, CM], f32, kind="ExternalOutput").ap()

    with tile.TileContext(nc) as tc, ExitStack() as ctx:
        emit_kernel(nc, tc, ctx, (qh, ql, kh, kl, v, mask, ones, out), b_core)

    nc.compile()
    return nc


def host_prep(query, key, value):
    """Full inputs -> per-core in_maps (host-side layout prep + sharding)."""
    q = np.ascontiguousarray(np.asarray(query, dtype=np.float32)).reshape(
        B_TOTAL, NQ, C
    )
    k = np.ascontiguousarray(np.asarray(key, dtype=np.float32)).reshape(
        B_TOTAL, NQ, C
    )
    v = np.ascontiguousarray(np.asarray(value, dtype=np.float32)).reshape(
        B_TOTAL, NQ, CM
    )
    qt = np.ascontiguousarray(q.transpose(0, 2, 1))  # [B, C, NQ]
    kt = np.ascontiguousarray(k.transpose(0, 2, 1))
    bft = ml_dtypes.bfloat16
    qth = qt.astype(bft)
    qtl = (qt - qth.astype(np.float32)).astype(bft)
    kth = kt.astype(bft)
    ktl = (kt - kth.astype(np.float32)).astype(bft)
    mask_np = np.triu(np.ones((128, 128), dtype=np.float32), k=1)
    ones_np = np.ones((128, 1), dtype=np.float32)

    b_core = B_TOTAL // N_CORES
    in_maps = []
    for cidx in range(N_CORES):
        sl = slice(b_core * cidx, b_core * (cidx + 1))
        in_maps.append(
            {
                "qh": np.ascontiguousarray(qth[sl]),
                "ql": np.ascontiguousarray(qtl[sl]),
                "kh": np.ascontiguousarray(kth[sl]),
                "kl": np.ascontiguousarray(ktl[sl]),
                "v": np.ascontiguousarray(v[sl]),
                "mask": mask_np,
                "ones": ones_np,
            }
        )
    return in_maps


def kernel(query, key, value):
    b_core = B_TOTAL // N_CORES
    if "nc" not in _cache:
        _cache["nc"] = build(b_core)
    nc = _cache["nc"]
    in_maps = host_prep(query, key, value)
    res = run_bass_kernel_spmd(
        nc, in_maps, core_ids=list(range(N_CORES)), trace=False
    )
    out = np.concatenate([r["out"] for r in res.results], axis=0)
    return out.reshape(B_TOTAL, 32, 32, CM).astype(np.float32)


if __name__ == "__main__":
    rng = np.random.default_rng(0)
    q = rng.standard_normal((B_TOTAL, 32, 32, C), dtype=np.float32)
    k = rng.standard_normal((B_TOTAL, 32, 32, C), dtype=np.float32)
    v = rng.standard_normal((B_TOTAL, 32, 32, CM), dtype=np.float32)
    o = kernel(query=q, key=k, value=v)
    print(o.shape, o.dtype)


# revision 22
# speedup vs baseline: 652.8469x; 652.8469x over previous
"""Trainium2 Bass kernel for nn_CausalAttentionLayer (sparse_attention).

Reference computes, per batch b (B=32, Nq=Nk=1024, C=128, CM=256):
    S = Q @ K^T                      # [1024, 1024], no 1/sqrt(d) scale
    P = softmax(S, axis=-1) * strict_lower_mask   # mask AFTER full-row softmax
    O = P @ V                        # [1024, 256]

Sharding: data-parallel over batch, 4 batches per core on 8 NeuronCores.

Device algorithm (per batch), in the transposed S^T = K Q^T layout (k on
partitions, q on free) so no on-device transposes are needed:
  - Q^T, K^T prepared on host as [C, Nq] bf16 hi/lo pairs;
    S^T block = Kh^T Qh + Kl^T Qh + Kh^T Ql (error ~2^-18, fp32 PSUM accum)
  - P^T = exp(S^T) on ScalarE in [128, 512] halves, written float32r to SBUF
  - l[q] = sum_k exp(S[q, k]) over ALL k via ones-matmuls into PSUM [1, 1024],
    then redistributed to r = 1/l in [128, 8] via 8 tiny transpose-matmuls
  - strict-upper mask on diagonal blocks (DVE), causal PV (36 of 64 block mms,
    f32r), O *= r (per-partition scalar), DMA out.

Scheduling: batch-level software pipeline — the PV phase of batch b is
emitted AFTER the S phase of batch b+1 so the PE never waits on the
exp -> l -> 1/l chain; l ones-matmuls run one block late (lag-1) so they
never wait on ACT.
"""

import sys
from contextlib import ExitStack

import numpy as np

sys.path.insert(0, "/opt/trn_rl_repo")

import ml_dtypes  # noqa: E402

import concourse.tile as tile  # noqa: E402
from concourse import bacc, mybir  # noqa: E402
from concourse.bass_utils import run_bass_kernel_spmd  # noqa: E402

N_CORES = 8
B_TOTAL = 32
NQ = 1024
C = 128
CM = 256
NBLK = NQ // 128  # 8

_cache = {}

f32 = mybir.dt.float32
f32r = mybir.dt.float32r
bf16 = mybir.dt.bfloat16


def emit_kernel(nc, tc, ctx, aps, b_core):
    qh, ql, kh, kl, v, mask, ones, out = aps
    const_pool = ctx.enter_context(tc.tile_pool(name="const", bufs=1))
    qk_pool = ctx.enter_context(tc.tile_pool(name="qk", bufs=2))
    v_pool = ctx.enter_context(tc.tile_pool(name="vp", bufs=2))
    pt_pool = ctx.enter_context(tc.tile_pool(name="pt", bufs=16))
    ptm_pool = ctx.enter_context(tc.tile_pool(name="ptm", bufs=2))
    l_pool = ctx.enter_context(tc.tile_pool(name="lsb", bufs=2))
    r_pool = ctx.enter_context(tc.tile_pool(name="rsb", bufs=2))
    o_pool = ctx.enter_context(tc.tile_pool(name="osb", bufs=2))
    acc_pool = ctx.enter_context(tc.tile_pool(name="acc", bufs=6))
    ps_s = ctx.enter_context(tc.tile_pool(name="ps_s", bufs=4, space="PSUM"))
    ps_o = ctx.enter_context(tc.tile_pool(name="ps_o", bufs=2, space="PSUM"))
    ps_l = ctx.enter_context(tc.tile_pool(name="ps_l", bufs=1, space="PSUM"))

    mask_sb = const_pool.tile([128, 128], f32r)
    nc.sync.dma_start(mask_sb[:], mask)
    ones_sb = const_pool.tile([128, 1], f32r)
    nc.sync.dma_start(ones_sb[:], ones)
    onesf_sb = const_pool.tile([1, 1], f32)
    nc.vector.memset(onesf_sb[:], 1.0)

    def emit_s_phase(b):
        st = {}
        # packed bf16 [qh | ql | kh | kl] input, loaded as 4 DMAs so the
        # transfers spread across DMA queues
        qk_sb = qk_pool.tile([C, 4 * NQ], bf16, tag="qk")
        for quad in (0, 2, 1, 3):  # qh, kh first — the first matmul's inputs
            nc.sync.dma_start(
                qk_sb[:, NQ * quad : NQ * (quad + 1)],
                qh[b, :, NQ * quad : NQ * (quad + 1)],
            )
        qh_sb = qk_sb[:, 0 * NQ : 1 * NQ]
        ql_sb = qk_sb[:, 1 * NQ : 2 * NQ]
        kh_sb = qk_sb[:, 2 * NQ : 3 * NQ]
        kl_sb = qk_sb[:, 3 * NQ : 4 * NQ]
        # V loaded as 8 per-block DMAs (queue parallelism)
        v_all = v_pool.tile([128, NBLK * CM], f32r, tag="v")
        for i in range(NBLK):
            nc.sync.dma_start(
                v_all[:, CM * i : CM * (i + 1)],
                v[b, 128 * i : 128 * (i + 1), :],
            )
        st["v"] = [v_all[:, CM * i : CM * (i + 1)] for i in range(NBLK)]

        psl = ps_l.tile([1, NQ], f32)
        st["psl"] = psl
        st["pt"] = []
        st["acc"] = []

        def pair_add(p, eng):
            # acc_p = pt[2p] + pt[2p+1] on an engine with slack (DVE/GpSimd)
            acc = acc_pool.tile([128, NQ], f32r, tag="acc")
            eng.tensor_add(acc[:], st["pt"][2 * p][:], st["pt"][2 * p + 1][:])
            st["acc"].append(acc)

        def l_mms(p):
            for h in (0, 1):
                nc.tensor.matmul(
                    psl[:, 512 * h : 512 * (h + 1)],
                    ones_sb[:],
                    st["acc"][p][:, 512 * h : 512 * (h + 1)],
                    start=(p == 0),
                    stop=(p == 3),
                    skip_group_check=True,
                )

        for i in range(NBLK):
            kslc = slice(128 * i, 128 * (i + 1))
            pt_i = pt_pool.tile([128, NQ], f32r, tag="pt")
            for h in (0, 1):
                qslc = slice(512 * h, 512 * (h + 1))
                s_ps = ps_s.tile([128, 512], f32, tag="s")
                terms = ((kh_sb, qh_sb), (kl_sb, qh_sb), (kh_sb, ql_sb))
                for t, (kt_, qt_) in enumerate(terms):
                    nc.tensor.matmul(
                        s_ps[:], kt_[:, kslc], qt_[:, qslc],
                        start=(t == 0), stop=(t == 2),
                    )
                nc.scalar.activation(
                    pt_i[:, qslc], s_ps[:], mybir.ActivationFunctionType.Exp
                )
            st["pt"].append(pt_i)
            # lagged pair-adds on DVE / GpSimd + lagged l-mms
            if i == 2:
                pair_add(0, nc.vector)
            elif i == 4:
                pair_add(1, nc.gpsimd)
                l_mms(0)
            elif i == 6:
                pair_add(2, nc.gpsimd)
                l_mms(1)
        return st

    def emit_l_tail(st):
        # final pair-add + l-mms + l copy; emitted after the previous
        # batch's PV phase so the PE has filler while exp(7)/adds finish
        pair_add_tail = acc_pool.tile([128, NQ], f32r, tag="acc")
        nc.gpsimd.tensor_add(
            pair_add_tail[:], st["pt"][6][:], st["pt"][7][:]
        )
        st["acc"].append(pair_add_tail)
        for p in (2, 3):
            for h in (0, 1):
                nc.tensor.matmul(
                    st["psl"][:, 512 * h : 512 * (h + 1)],
                    ones_sb[:],
                    st["acc"][p][:, 512 * h : 512 * (h + 1)],
                    start=False,
                    stop=(p == 3),
                    skip_group_check=True,
                )
        l_sb = l_pool.tile([1, NQ], f32, tag="l")
        nc.scalar.copy(l_sb[:], st["psl"][:])
        st["l_sb"] = l_sb

    def emit_pv_phase(st):
        # diagonal-block masks (DVE) — inputs were ready long ago
        ptm = ptm_pool.tile([128, NQ], f32r, tag="ptm")
        for i in range(NBLK):
            kslc = slice(128 * i, 128 * (i + 1))
            nc.vector.tensor_mul(ptm[:, kslc], st["pt"][i][:, kslc], mask_sb[:])
        # l [1, 1024] -> [128, 8] partition-spread via 8 small DMAs
        # (lt[p, j] = l[128j + p]), then reciprocal on DVE
        lt_sb = r_pool.tile([128, NBLK], f32, tag="lt")
        for j in range(NBLK):
            nc.sync.dma_start(
                lt_sb[:, j : j + 1],
                st["l_sb"][0:1, 128 * j : 128 * (j + 1)],
            )
        r_sb = r_pool.tile([128, NBLK], f32, tag="r")
        nc.vector.reciprocal(r_sb[:], lt_sb[:])

        o_all = o_pool.tile([128, NBLK * CM], f32, tag="o_sb")
        for j in range(NBLK):
            jslc = slice(128 * j, 128 * (j + 1))
            o_ps = ps_o.tile([128, CM], f32, tag="o")
            for i in range(j + 1):
                lhsT = ptm[:, jslc] if i == j else st["pt"][i][:, jslc]
                nc.tensor.matmul(
                    o_ps[:], lhsT, st["v"][i],
                    start=(i == 0), stop=(i == j),
                )
            nc.vector.tensor_scalar_mul(
                o_all[:, CM * j : CM * (j + 1)], o_ps[:], r_sb[:, j : j + 1]
            )
            nc.sync.dma_start(
                out[st["b"], jslc, :], o_all[:, CM * j : CM * (j + 1)]
            )


    prev = None
    for b in range(b_core):
        st = emit_s_phase(b)
        st["b"] = b
        if prev is not None:
            emit_pv_phase(prev)
        emit_l_tail(st)
        prev = st
    emit_pv_phase(prev)


def declare_io(nc, b_core):
    qk = nc.dram_tensor(
        "qk", [b_core, C, 4 * NQ], bf16, kind="ExternalInput"
    ).ap()
    v = nc.dram_tensor("v", [b_core, NQ, CM], f32r, kind="ExternalInput").ap()
    mask = nc.dram_tensor("mask", [128, 128], f32r, kind="ExternalInput").ap()
    ones = nc.dram_tensor("ones", [128, 1], f32r, kind="ExternalInput").ap()
    out = nc.dram_tensor("out", [b_core, NQ, CM], f32, kind="ExternalOutput").ap()
    return (qk, None, None, None, v, mask, ones, out)


def build(b_core):
    """Build + compile the per-core Bass program processing b_core batches."""
    nc = bacc.Bacc(
        "TRN2", target_bir_lowering=False, debug=False, num_devices=N_CORES
    )
    aps = declare_io(nc, b_core)
    with tile.TileContext(nc) as tc, ExitStack() as ctx:
        emit_kernel(nc, tc, ctx, aps, b_core)

    nc.compile()
    return nc


def host_prep(query, key, value):
    """Full inputs -> per-core in_maps (host-side layout prep + sharding)."""
    q = np.ascontiguousarray(np.asarray(query, dtype=np.float32)).reshape(
        B_TOTAL, NQ, C
    )
    k = np.ascontiguousarray(np.asarray(key, dtype=np.float32)).reshape(
        B_TOTAL, NQ, C
    )
    v = np.ascontiguousarray(np.asarray(value, dtype=np.float32)).reshape(
        B_TOTAL, NQ, CM
    )
    qt = np.ascontiguousarray(q.transpose(0, 2, 1))  # [B, C, NQ]
    kt = np.ascontiguousarray(k.transpose(0, 2, 1))
    bft = ml_dtypes.bfloat16
    qth = qt.astype(bft)
    qtl = (qt - qth.astype(np.float32)).astype(bft)
    kth = kt.astype(bft)
    ktl = (kt - kth.astype(np.float32)).astype(bft)
    mask_np = np.triu(np.ones((128, 128), dtype=np.float32), k=1)
    ones_np = np.ones((128, 1), dtype=np.float32)

    qk = np.ascontiguousarray(
        np.concatenate([qth, qtl, kth, ktl], axis=2)
    )  # [B, C, 4*NQ] bf16

    b_core = B_TOTAL // N_CORES
    in_maps = []
    for cidx in range(N_CORES):
        sl = slice(b_core * cidx, b_core * (cidx + 1))
        in_maps.append(
            {
                "qk": np.ascontiguousarray(qk[sl]),
                "v": np.ascontiguousarray(v[sl]),
                "mask": mask_np,
                "ones": ones_np,
            }
        )
    return in_maps


def kernel(query, key, value):
    b_core = B_TOTAL // N_CORES
    if "nc" not in _cache:
        _cache["nc"] = build(b_core)
    nc = _cache["nc"]
    in_maps = host_prep(query, key, value)
    res = run_bass_kernel_spmd(
        nc, in_maps, core_ids=list(range(N_CORES)), trace=False
    )
    out = np.concatenate([r["out"] for r in res.results], axis=0)
    return out.reshape(B_TOTAL, 32, 32, CM).astype(np.float32)


if __name__ == "__main__":
    rng = np.random.default_rng(0)
    q = rng.standard_normal((B_TOTAL, 32, 32, C), dtype=np.float32)
    k = rng.standard_normal((B_TOTAL, 32, 32, C), dtype=np.float32)
    v = rng.standard_normal((B_TOTAL, 32, 32, CM), dtype=np.float32)
    o = kernel(query=q, key=k, value=v)
    print(o.shape, o.dtype)


# revision 23
# speedup vs baseline: 686.5988x; 1.0517x over previous
"""Trainium2 Bass kernel for nn_CausalAttentionLayer (sparse_attention).

Reference computes, per batch b (B=32, Nq=Nk=1024, C=128, CM=256):
    S = Q @ K^T                      # [1024, 1024], no 1/sqrt(d) scale
    P = softmax(S, axis=-1) * strict_lower_mask   # mask AFTER full-row softmax
    O = P @ V                        # [1024, 256]

Sharding: data-parallel over batch, 4 batches per core on 8 NeuronCores.

Device algorithm (per batch), in the transposed S^T = K Q^T layout (k on
partitions, q on free) so no on-device transposes are needed:
  - Q^T, K^T prepared on host as [C, Nq] bf16 hi/lo pairs;
    S^T block = Kh^T Qh + Kl^T Qh + Kh^T Ql (error ~2^-18, fp32 PSUM accum)
  - P^T = exp(S^T) on ScalarE in [128, 512] halves, written float32r to SBUF
  - l[q] = sum_k exp(S[q, k]) over ALL k: pt tiles pair-summed on DVE/GpSimd,
    then ones-matmuls into PSUM [1, 1024], spread to [128, 8] via 8 small DMAs,
    reciprocal on DVE
  - strict-upper mask on diagonal blocks (DVE), causal PV (36 of 64 block mms,
    f32r), O *= r (per-partition scalar), DMA out.

Scheduling: batch-level software pipeline — the PV phase of batch b is
emitted AFTER the S phase of batch b+1 so the PE never waits on the
exp -> l -> 1/l chain; l pair-adds and ones-matmuls run lagged so they
never stall the PE.
"""

import sys
from contextlib import ExitStack

import numpy as np

sys.path.insert(0, "/opt/trn_rl_repo")

import ml_dtypes  # noqa: E402

import concourse.tile as tile  # noqa: E402
from concourse import bacc, mybir  # noqa: E402
from concourse.bass_utils import run_bass_kernel_spmd  # noqa: E402

N_CORES = 8
B_TOTAL = 32
NQ = 1024
C = 128
CM = 256
NBLK = NQ // 128  # 8

_cache = {}

f32 = mybir.dt.float32
f32r = mybir.dt.float32r
bf16 = mybir.dt.bfloat16


def emit_kernel(nc, tc, ctx, aps, b_core):
    qh, ql, kh, kl, v, mask, ones, out = aps
    const_pool = ctx.enter_context(tc.tile_pool(name="const", bufs=1))
    qk_pool = ctx.enter_context(tc.tile_pool(name="qk", bufs=2))
    v_pool = ctx.enter_context(tc.tile_pool(name="vp", bufs=2))
    pt_pool = ctx.enter_context(tc.tile_pool(name="pt", bufs=16))
    ptm_pool = ctx.enter_context(tc.tile_pool(name="ptm", bufs=2))
    l_pool = ctx.enter_context(tc.tile_pool(name="lsb", bufs=2))
    r_pool = ctx.enter_context(tc.tile_pool(name="rsb", bufs=2))
    o_pool = ctx.enter_context(tc.tile_pool(name="osb", bufs=2))
    acc_pool = ctx.enter_context(tc.tile_pool(name="acc", bufs=6))
    ps_s = ctx.enter_context(tc.tile_pool(name="ps_s", bufs=4, space="PSUM"))
    ps_o = ctx.enter_context(tc.tile_pool(name="ps_o", bufs=2, space="PSUM"))
    ps_l = ctx.enter_context(tc.tile_pool(name="ps_l", bufs=1, space="PSUM"))

    mask_sb = const_pool.tile([128, 128], f32r)
    nc.sync.dma_start(mask_sb[:], mask)
    ones_sb = const_pool.tile([128, 1], f32r)
    nc.sync.dma_start(ones_sb[:], ones)
    onesf_sb = const_pool.tile([1, 1], f32)
    nc.vector.memset(onesf_sb[:], 1.0)

    def emit_s_phase(b):
        st = {}
        # packed bf16 [qh | ql | kh | kl] input, loaded as 4 DMAs so the
        # transfers spread across DMA queues
        qk_sb = qk_pool.tile([C, 4 * NQ], bf16, tag="qk")
        for quad in (0, 2, 1, 3):  # qh, kh first — the first matmul's inputs
            nc.sync.dma_start(
                qk_sb[:, NQ * quad : NQ * (quad + 1)],
                qh[b, :, NQ * quad : NQ * (quad + 1)],
            )
        qh_sb = qk_sb[:, 0 * NQ : 1 * NQ]
        ql_sb = qk_sb[:, 1 * NQ : 2 * NQ]
        kh_sb = qk_sb[:, 2 * NQ : 3 * NQ]
        kl_sb = qk_sb[:, 3 * NQ : 4 * NQ]
        # V loaded as 8 per-block DMAs (queue parallelism)
        v_all = v_pool.tile([128, NBLK * CM], f32r, tag="v")
        for i in range(NBLK):
            nc.sync.dma_start(
                v_all[:, CM * i : CM * (i + 1)],
                v[b, 128 * i : 128 * (i + 1), :],
            )
        st["v"] = [v_all[:, CM * i : CM * (i + 1)] for i in range(NBLK)]

        psl = ps_l.tile([1, NQ], f32)
        st["psl"] = psl
        st["pt"] = []
        st["acc"] = []

        def pair_add(p, eng):
            # acc_p = pt[2p] + pt[2p+1] on an engine with slack (DVE/GpSimd)
            acc = acc_pool.tile([128, NQ], f32r, tag="acc")
            eng.tensor_add(acc[:], st["pt"][2 * p][:], st["pt"][2 * p + 1][:])
            st["acc"].append(acc)

        def l_mms(p):
            for h in (0, 1):
                nc.tensor.matmul(
                    psl[:, 512 * h : 512 * (h + 1)],
                    ones_sb[:],
                    st["acc"][p][:, 512 * h : 512 * (h + 1)],
                    start=(p == 0),
                    stop=(p == 3),
                    skip_group_check=True,
                )

        for i in range(NBLK):
            kslc = slice(128 * i, 128 * (i + 1))
            pt_i = pt_pool.tile([128, NQ], f32r, tag="pt")
            for h in (0, 1):
                qslc = slice(512 * h, 512 * (h + 1))
                s_ps = ps_s.tile([128, 512], f32, tag="s")
                terms = ((kh_sb, qh_sb), (kl_sb, qh_sb), (kh_sb, ql_sb))
                for t, (kt_, qt_) in enumerate(terms):
                    nc.tensor.matmul(
                        s_ps[:], kt_[:, kslc], qt_[:, qslc],
                        start=(t == 0), stop=(t == 2),
                    )
                nc.scalar.activation(
                    pt_i[:, qslc], s_ps[:], mybir.ActivationFunctionType.Exp
                )
            st["pt"].append(pt_i)
            # lagged pair-adds on DVE / GpSimd + lagged l-mms
            if i == 2:
                pair_add(0, nc.vector)
            elif i == 4:
                pair_add(1, nc.gpsimd)
                l_mms(0)
            elif i == 6:
                pair_add(2, nc.gpsimd)
                l_mms(1)
        return st

    def emit_l_tail(st):
        # final pair-add + l-mms + l copy; emitted after the previous
        # batch's PV phase so the PE has filler while exp(7)/adds finish
        pair_add_tail = acc_pool.tile([128, NQ], f32r, tag="acc")
        nc.gpsimd.tensor_add(
            pair_add_tail[:], st["pt"][6][:], st["pt"][7][:]
        )
        st["acc"].append(pair_add_tail)
        for p in (2, 3):
            for h in (0, 1):
                nc.tensor.matmul(
                    st["psl"][:, 512 * h : 512 * (h + 1)],
                    ones_sb[:],
                    st["acc"][p][:, 512 * h : 512 * (h + 1)],
                    start=False,
                    stop=(p == 3),
                    skip_group_check=True,
                )
        l_sb = l_pool.tile([1, NQ], f32, tag="l")
        nc.scalar.copy(l_sb[:], st["psl"][:])
        st["l_sb"] = l_sb

    def emit_pv_phase(st):
        # diagonal-block masks (DVE) — inputs were ready long ago
        ptm = ptm_pool.tile([128, NQ], f32r, tag="ptm")
        for i in range(NBLK):
            kslc = slice(128 * i, 128 * (i + 1))
            nc.vector.tensor_mul(ptm[:, kslc], st["pt"][i][:, kslc], mask_sb[:])
        # l [1, 1024] -> [128, 8] partition-spread via 8 small DMAs
        # (lt[p, j] = l[128j + p]), then reciprocal on DVE
        lt_sb = r_pool.tile([128, NBLK], f32, tag="lt")
        for j in range(NBLK):
            nc.sync.dma_start(
                lt_sb[:, j : j + 1],
                st["l_sb"][0:1, 128 * j : 128 * (j + 1)],
            )
        r_sb = r_pool.tile([128, NBLK], f32, tag="r")
        nc.vector.reciprocal(r_sb[:], lt_sb[:])

        o_all = o_pool.tile([128, NBLK * CM], f32, tag="o_sb")
        for j in range(NBLK):
            jslc = slice(128 * j, 128 * (j + 1))
            o_ps = ps_o.tile([128, CM], f32, tag="o")
            for i in range(j + 1):
                lhsT = ptm[:, jslc] if i == j else st["pt"][i][:, jslc]
                nc.tensor.matmul(
                    o_ps[:], lhsT, st["v"][i],
                    start=(i == 0), stop=(i == j),
                )
            nc.vector.tensor_scalar_mul(
                o_all[:, CM * j : CM * (j + 1)], o_ps[:], r_sb[:, j : j + 1]
            )
            nc.sync.dma_start(
                out[st["b"], jslc, :], o_all[:, CM * j : CM * (j + 1)]
            )


    prev = None
    for b in range(b_core):
        st = emit_s_phase(b)
        st["b"] = b
        if prev is not None:
            emit_pv_phase(prev)
        emit_l_tail(st)
        prev = st
    emit_pv_phase(prev)


def declare_io(nc, b_core):
    qk = nc.dram_tensor(
        "qk", [b_core, C, 4 * NQ], bf16, kind="ExternalInput"
    ).ap()
    v = nc.dram_tensor("v", [b_core, NQ, CM], f32r, kind="ExternalInput").ap()
    mask = nc.dram_tensor("mask", [128, 128], f32r, kind="ExternalInput").ap()
    ones = nc.dram_tensor("ones", [128, 1], f32r, kind="ExternalInput").ap()
    out = nc.dram_tensor("out", [b_core, NQ, CM], f32, kind="ExternalOutput").ap()
    return (qk, None, None, None, v, mask, ones, out)


def build(b_core):
    """Build + compile the per-core Bass program processing b_core batches."""
    nc = bacc.Bacc(
        "TRN2", target_bir_lowering=False, debug=False, num_devices=N_CORES
    )
    aps = declare_io(nc, b_core)
    with tile.TileContext(nc) as tc, ExitStack() as ctx:
        emit_kernel(nc, tc, ctx, aps, b_core)

    nc.compile()
    return nc


def host_prep(query, key, value):
    """Full inputs -> per-core in_maps (host-side layout prep + sharding)."""
    q = np.ascontiguousarray(np.asarray(query, dtype=np.float32)).reshape(
        B_TOTAL, NQ, C
    )
    k = np.ascontiguousarray(np.asarray(key, dtype=np.float32)).reshape(
        B_TOTAL, NQ, C
    )
    v = np.ascontiguousarray(np.asarray(value, dtype=np.float32)).reshape(
        B_TOTAL, NQ, CM
    )
    qt = np.ascontiguousarray(q.transpose(0, 2, 1))  # [B, C, NQ]
    kt = np.ascontiguousarray(k.transpose(0, 2, 1))
    bft = ml_dtypes.bfloat16
    qth = qt.astype(bft)
    qtl = (qt - qth.astype(np.float32)).astype(bft)
    kth = kt.astype(bft)
    ktl = (kt - kth.astype(np.float32)).astype(bft)
    mask_np = np.triu(np.ones((128, 128), dtype=np.float32), k=1)
    ones_np = np.ones((128, 1), dtype=np.float32)

    qk = np.ascontiguousarray(
        np.concatenate([qth, qtl, kth, ktl], axis=2)
    )  # [B, C, 4*NQ] bf16

    b_core = B_TOTAL // N_CORES
    in_maps = []
    for cidx in range(N_CORES):
        sl = slice(b_core * cidx, b_core * (cidx + 1))
        in_maps.append(
            {
                "qk": np.ascontiguousarray(qk[sl]),
                "v": np.ascontiguousarray(v[sl]),
                "mask": mask_np,
                "ones": ones_np,
            }
        )
    return in_maps


def kernel(query, key, value):
    b_core = B_TOTAL // N_CORES
    if "nc" not in _cache:
        _cache["nc"] = build(b_core)
    nc = _cache["nc"]
    in_maps = host_prep(query, key, value)
    res = run_bass_kernel_spmd(
        nc, in_maps, core_ids=list(range(N_CORES)), trace=False
    )
    out = np.concatenate([r["out"] for r in res.results], axis=0)
    return out.reshape(B_TOTAL, 32, 32, CM).astype(np.float32)


if __name__ == "__main__":
    rng = np.random.default_rng(0)
    q = rng.standard_normal((B_TOTAL, 32, 32, C), dtype=np.float32)
    k = rng.standard_normal((B_TOTAL, 32, 32, C), dtype=np.float32)
    v = rng.standard_normal((B_TOTAL, 32, 32, CM), dtype=np.float32)
    o = kernel(query=q, key=k, value=v)
    print(o.shape, o.dtype)
